# revision 19
# baseline (speedup 1.0000x reference)
"""Trainium2 Bass kernel for AttentionAggregation (GNN message passing).

Computes  y = segment_sum(alpha[:,None] * (x @ W.T)[idx_j], idx_i, n_atoms)

Key algebraic identity: the W transform commutes with the (linear) segment
sum, so we aggregate raw x rows first and apply W.T at the end on the
(much smaller) per-core output window:

    y = segment_sum(alpha * x[idx_j], idx_i) @ W.T

Distribution: edges are partitioned across 8 cores by contiguous atom
ranges of idx_i (idx_i is sorted), so each core owns a disjoint atom
window of n_atoms/8 output rows -> pure concatenation, no all-reduce.

Per-core device pipeline:
  1. SWDGE dma_gather of x rows (bf16, 256B each) from DRAM by idx_j,
     128 edges per "chunk", GB chunks per call. dma_gather indices are
     int16, so the x table is split at row 32768 into lo/hi halves and
     each core's edge stream is partitioned into a lo pass and a hi pass
     (both still sorted by idx_i).
  2. Build a sparse selector matrix M[e, a] = alpha_e * (idx_i_local[e] ==
     anchor_c + a) on the vector engines (iota + is_equal + mul).
  3. PE matmul  psum[f, a] += g[e, f]^T . M[e, a]  accumulates segment
     sums into a sliding PSUM window of 512 atoms (idx_i sorted => chunk
     anchors advance monotonically within a pass).
  4. Window drain: psum added into an SBUF s^T [128, atoms] accumulator.
  5. Final transform y^T = W @ s^T via PE matmuls (f32), DMA out.

All index manipulation (sharding, lo/hi split, padding, chunk anchors)
happens on the host; the floating point work (gather, scale, segment
sum, W transform) runs on the NeuronCores.
"""

import sys

for _p in ("/opt/trn_rl_repo", "/root/.axon_site/_ro/trn_rl_repo"):
    if _p not in sys.path:
        sys.path.append(_p)

import numpy as np
import ml_dtypes

import concourse.bass as bass
import concourse.bacc as bacc
import concourse.mybir as mybir
from concourse.bass_utils import run_bass_kernel_spmd
from concourse.tile import TileContext

BF16 = ml_dtypes.bfloat16

NCORES = 8
CHUNK = 128          # edges per PE matmul (contraction width)
GB = 32              # chunks per M-build op / g tile
GCALL = 8            # chunks per dma_gather call (1024 idx; >=2048 kills HW)
PSUM_W = 512         # psum window width (one f32 bank)
LO = 32768           # int16 index limit for dma_gather


def _ceil_to(v, m):
    return -(-v // m) * m


def _pass_plan(streams, nch):
    """streams: per-core (j_local, i_local, alpha) for one pass (i-sorted).
    Pads every core to nch chunks; returns J, IL, AL [NCORES, nch*128] and
    anchors (len nch), spread."""
    npos = nch * CHUNK
    J = np.zeros((NCORES, npos), np.int32)
    IL = np.zeros((NCORES, npos), np.int32)
    AL = np.zeros((NCORES, npos), np.float32)
    for m, (j, il, al) in enumerate(streams):
        n = len(j)
        J[m, :n] = j
        IL[m, :n] = il
        AL[m, :n] = al
        IL[m, n:] = il[-1] if n > 0 else 0
    ILc = IL.reshape(NCORES, nch, CHUNK)
    anchors = ILc.min(axis=2).min(axis=0).astype(np.int64)
    anchors = np.maximum.accumulate(anchors)
    spread = int((ILc.max(axis=2).max(axis=0) - anchors).max()) + 1
    return J, IL, AL, anchors, spread


def _regions_for(anchors, A, base_c):
    regions = []
    if len(anchors) == 0:
        return regions
    c0, r0 = 0, int(anchors[0])
    for c in range(len(anchors)):
        if int(anchors[c]) + A - r0 > PSUM_W:
            regions.append((base_c + c0, base_c + c, r0))
            c0, r0 = c, int(anchors[c])
    regions.append((base_c + c0, base_c + len(anchors), r0))
    return regions


def _make_plan(x, alpha_ij, idx_i, idx_j, W):
    n_atoms, F = x.shape
    assert F == 128 and n_atoms % NCORES == 0
    apc = n_atoms // NCORES  # atoms per core

    idx_i = np.asarray(idx_i)
    idx_j = np.asarray(idx_j)
    alpha_ij = np.asarray(alpha_ij, dtype=np.float32)

    bounds = np.searchsorted(idx_i, np.arange(NCORES + 1) * apc)
    lo_streams, hi_streams = [], []
    nch_lo = nch_hi = 0
    for m in range(NCORES):
        e0, e1 = int(bounds[m]), int(bounds[m + 1])
        j = idx_j[e0:e1].astype(np.int32)
        il = (idx_i[e0:e1] - m * apc).astype(np.int32)
        al = alpha_ij[e0:e1]
        sel = j < LO
        lo_streams.append((j[sel], il[sel], al[sel]))
        hi_streams.append((j[~sel] - LO, il[~sel], al[~sel]))
        nch_lo = max(nch_lo, len(lo_streams[-1][0]))
        nch_hi = max(nch_hi, len(hi_streams[-1][0]))

    nch_lo = _ceil_to(_ceil_to(nch_lo, CHUNK) // CHUNK, GB) if nch_lo else 0
    nch_hi = _ceil_to(_ceil_to(nch_hi, CHUNK) // CHUNK, GB) if nch_hi else 0
    nch = nch_lo + nch_hi

    Jl, ILl, ALl, anch_lo, spread_lo = _pass_plan(lo_streams, nch_lo)
    if nch_hi:
        Jh, ILh, ALh, anch_hi, spread_hi = _pass_plan(hi_streams, nch_hi)
    else:
        Jh = np.zeros((NCORES, 0), np.int32)
        ILh = np.zeros((NCORES, 0), np.int32)
        ALh = np.zeros((NCORES, 0), np.float32)
        anch_hi, spread_hi = np.zeros(0, np.int64), 1

    A_lo = _ceil_to(spread_lo, 8)
    A_hi = _ceil_to(spread_hi, 8) if nch_hi else 8
    A = max(A_lo, A_hi)
    assert A <= PSUM_W, f"chunk atom spread {A} exceeds psum window {PSUM_W}"

    regions = (_regions_for(anch_lo, A_lo, 0) +
               _regions_for(anch_hi, A_hi, nch_lo))
    anchors = np.concatenate([anch_lo, anch_hi])

    out_w = _ceil_to(apc, PSUM_W)
    st_w = max(max(r0 for _, _, r0 in regions) + PSUM_W, out_w)

    J = np.concatenate([Jl, Jh], axis=1)
    IL = np.concatenate([ILl, ILh], axis=1)
    AL = np.concatenate([ALl, ALh], axis=1)
    idxl = IL - np.repeat(anchors[None, :], CHUNK, axis=0).T.reshape(1, nch * CHUNK)

    def to2d(arr, dt):
        # position (c*128 + p) -> [p, c]
        return np.ascontiguousarray(arr.reshape(-1, CHUNK).T).astype(dt)

    def wrap16(arr):
        # position k -> partition k%16 (replicated x8), word k//16
        w16 = np.ascontiguousarray(arr.reshape(-1, 16).T)  # [16, npos/16]
        return np.tile(w16, (8, 1)).astype(np.int16)

    in_maps = []
    xb = np.asarray(x, dtype=np.float32).astype(BF16)
    wt = np.ascontiguousarray(np.asarray(W, dtype=np.float32).T)
    iota = np.tile(np.arange(A, dtype=np.float32).astype(BF16), (128, 1))
    for m in range(NCORES):
        in_maps.append({
            "xb": xb,
            "gidx": wrap16(J[m]),
            "idxl": to2d(idxl[m].astype(np.float32), BF16),
            "alp": to2d(AL[m], BF16),
            "iota": iota,
            "wt": wt,
        })

    return dict(
        n_atoms=n_atoms, F=F, apc=apc, nch=nch, nch_lo=nch_lo, A=A,
        A_lo=A_lo, A_hi=A_hi,
        anchors=[int(a) for a in anchors], regions=regions,
        out_w=out_w, st_w=st_w, in_maps=in_maps,
    )


def _build_nc(plan, repeat=1, parts=("gather", "m", "pe"), hw_loop=0):
    do_gather = "gather" in parts
    do_m = "m" in parts
    do_pe = "pe" in parts
    F = plan["F"]
    nch, nch_lo, A = plan["nch"], plan["nch_lo"], plan["A"]
    regions, anchors = plan["regions"], plan["anchors"]
    out_w, st_w = plan["out_w"], plan["st_w"]
    n_atoms = plan["n_atoms"]
    f32 = mybir.dt.float32
    bf16 = mybir.dt.bfloat16
    WPC = CHUNK // 16  # gidx words per chunk

    nc = bacc.Bacc("TRN2", target_bir_lowering=False, debug=False,
                   num_devices=NCORES, num_swdge_queues=4)

    xb_d = nc.dram_tensor("xb", [n_atoms, F], bf16, kind="ExternalInput")
    gidx_d = nc.dram_tensor("gidx", [128, nch * WPC], mybir.dt.int16,
                            kind="ExternalInput")
    idxl_d = nc.dram_tensor("idxl", [128, nch], bf16, kind="ExternalInput")
    alp_d = nc.dram_tensor("alp", [128, nch], bf16, kind="ExternalInput")
    iota_d = nc.dram_tensor("iota", [128, A], bf16, kind="ExternalInput")
    wt_d = nc.dram_tensor("wt", [F, F], f32, kind="ExternalInput")
    yt_d = nc.dram_tensor("yT", [128, out_w], f32, kind="ExternalOutput")

    region_start = {c0: (c0, c1, r0) for (c0, c1, r0) in regions}
    region_end = {c1 - 1: (c0, c1, r0) for (c0, c1, r0) in regions}

    with TileContext(nc) as tc:
        with (
            tc.tile_pool(name="const", bufs=1) as cpool,
            tc.tile_pool(name="gp", bufs=3) as gpool,
            tc.tile_pool(name="mp", bufs=3) as mpool,
            tc.tile_pool(name="pp", bufs=2, space="PSUM") as ppool,
            tc.tile_pool(name="yp", bufs=2, space="PSUM") as ypool,
            tc.tile_pool(name="yo", bufs=2) as yopool,
        ):
            gidx_s = cpool.tile([128, nch * WPC], mybir.dt.int16)
            idxl_s = cpool.tile([128, nch], bf16)
            alp_s = cpool.tile([128, nch], bf16)
            iota_s = cpool.tile([128, A], bf16)
            wt_s = cpool.tile([F, F], f32)
            st_s = cpool.tile([128, st_w], f32)

            nc.sync.dma_start(gidx_s[:, :], gidx_d[:, :])
            nc.sync.dma_start(idxl_s[:, :], idxl_d[:, :])
            nc.sync.dma_start(alp_s[:, :], alp_d[:, :])
            nc.sync.dma_start(iota_s[:, :], iota_d[:, :])
            nc.sync.dma_start(wt_s[:, :], wt_d[:, :])
            nc.vector.memset(st_s[:, :], 0.0)

            if not do_gather:
                g_const = gpool.tile([128, GB, F], bf16, tag="g_t")
                nc.vector.memset(g_const[:, :, :], 0.0)
            if not do_m:
                m_const = mpool.tile([128, GB, plan["A"]], bf16, tag="m_t")
                nc.vector.memset(m_const[:, :, :], 0.0)

            import contextlib
            loop_cm = (tc.For_i(0, hw_loop, 1) if hw_loop
                       else contextlib.nullcontext())
            with loop_cm:
                psum_t = None
                cur_r0 = 0
                for b in list(range(nch // GB)) * repeat:
                    if b * GB < nch_lo:
                        src = xb_d[0:min(n_atoms, LO), :]
                    else:
                        src = xb_d[LO:n_atoms, :]
                    Ab = plan["A_lo"] if b * GB < nch_lo else plan["A_hi"]
                    if do_gather:
                        g_t = gpool.tile([128, GB, F], bf16, tag="g_t")
                        for gc in range(GB // GCALL):
                            c0w = (b * GB + gc * GCALL) * WPC
                            nc.gpsimd.dma_gather(
                                g_t[:, gc * GCALL:(gc + 1) * GCALL, :], src,
                                gidx_s[:, c0w:c0w + GCALL * WPC],
                                GCALL * CHUNK, GCALL * CHUNK, F,
                                queue_num=gc % 4,
                            )
                    else:
                        g_t = g_const
                    if do_m:
                        m_t = mpool.tile([128, GB, Ab], bf16, tag="m_t")
                        idxl_b = (idxl_s[:, b * GB:(b + 1) * GB]
                                  .unsqueeze(2).broadcast_to((128, GB, Ab)))
                        alp_b = (alp_s[:, b * GB:(b + 1) * GB]
                                 .unsqueeze(2).broadcast_to((128, GB, Ab)))
                        iota_bb = (iota_s[:, :Ab]
                                   .unsqueeze(1).broadcast_to((128, GB, Ab)))
                        nc.any.tensor_tensor(m_t[:, :, :], iota_bb, idxl_b,
                                             op=mybir.AluOpType.is_equal)
                        nc.any.tensor_tensor(m_t[:, :, :], m_t[:, :, :],
                                             alp_b, op=mybir.AluOpType.mult)
                    else:
                        m_t = m_const

                    if do_pe:
                        for cl in range(GB):
                            c = b * GB + cl
                            if c in region_start:
                                _, _, cur_r0 = region_start[c]
                                psum_t = ppool.tile([128, PSUM_W], f32)
                                nc.vector.memset(psum_t[:, :], 0.0)
                            off = anchors[c] - cur_r0
                            nc.tensor.matmul(
                                psum_t[:, off:off + Ab],
                                g_t[:, cl, :],
                                m_t[:, cl, :Ab],
                                start=False,
                                stop=(c in region_end),
                                skip_group_check=True,
                            )
                            if c in region_end:
                                _, _, r0 = region_end[c]
                                nc.any.tensor_add(st_s[:, r0:r0 + PSUM_W],
                                                  psum_t[:, :],
                                                  st_s[:, r0:r0 + PSUM_W])

            for ob in range(out_w // PSUM_W):
                yp_t = ypool.tile([128, PSUM_W], f32)
                nc.tensor.matmul(
                    yp_t[:, :], wt_s[:, :],
                    st_s[:, ob * PSUM_W:(ob + 1) * PSUM_W],
                    start=True, stop=True,
                )
                yo_t = yopool.tile([128, PSUM_W], f32)
                nc.any.tensor_copy(yo_t[:, :], yp_t[:, :])
                nc.sync.dma_start(yt_d[:, ob * PSUM_W:(ob + 1) * PSUM_W],
                                  yo_t[:, :])

    nc.compile()
    return nc


def kernel(x, alpha_ij, idx_i, idx_j, W):
    plan = _make_plan(x, alpha_ij, idx_i, idx_j, W)
    nc = _build_nc(plan)
    res = run_bass_kernel_spmd(nc, plan["in_maps"], list(range(NCORES)))
    apc = plan["apc"]
    y = np.empty((plan["n_atoms"], plan["F"]), np.float32)
    for m in range(NCORES):
        y[m * apc:(m + 1) * apc] = res.results[m]["yT"][:, :apc].T
    return y


# revision 25
# speedup vs baseline: 9653.0807x; 9653.0807x over previous
"""Trainium2 Bass kernel for AttentionAggregation (GNN message passing).

Computes  y = segment_sum(alpha[:,None] * (x @ W.T)[idx_j], idx_i, n_atoms)

Key algebraic identity: the W transform commutes with the (linear) segment
sum, so we aggregate raw x rows first and apply W.T at the end on the
(much smaller) per-core output window:

    y = segment_sum(alpha * x[idx_j], idx_i) @ W.T

Distribution: edges are partitioned across 8 cores by contiguous atom
ranges of idx_i (idx_i is sorted), so each core owns a disjoint atom
window of n_atoms/8 output rows -> pure concatenation, no all-reduce.

Per-core device pipeline:
  1. SWDGE dma_gather of x rows (bf16, 256B each) from DRAM by idx_j,
     128 edges per "chunk", GB chunks per call. dma_gather indices are
     int16, so the x table is split at row 32768 into lo/hi halves and
     each core's edge stream is partitioned into a lo pass and a hi pass
     (both still sorted by idx_i).
  2. Build a sparse selector matrix M[e, a] = alpha_e * (idx_i_local[e] ==
     anchor_c + a) on the vector engines (iota + is_equal + mul).
  3. PE matmul  psum[f, a] += g[e, f]^T . M[e, a]  accumulates segment
     sums into a sliding PSUM window of 512 atoms (idx_i sorted => chunk
     anchors advance monotonically within a pass).
  4. Window drain: psum added into an SBUF s^T [128, atoms] accumulator.
  5. Final transform y^T = W @ s^T via PE matmuls (f32), DMA out.

All index manipulation (sharding, lo/hi split, padding, chunk anchors)
happens on the host; the floating point work (gather, scale, segment
sum, W transform) runs on the NeuronCores.
"""

import sys

for _p in ("/opt/trn_rl_repo", "/root/.axon_site/_ro/trn_rl_repo"):
    if _p not in sys.path:
        sys.path.append(_p)

import numpy as np
import ml_dtypes

import concourse.bass as bass
import concourse.bacc as bacc
import concourse.mybir as mybir
from concourse.bass_utils import run_bass_kernel_spmd
from concourse.tile import TileContext

BF16 = ml_dtypes.bfloat16

NCORES = 8
CHUNK = 128          # edges per PE matmul (contraction width)
GB = 16              # chunks per M-build op / g tile
GCALL = 8            # chunks per dma_gather call (1024 idx; >=2048 kills HW)
PSUM_W = 512         # psum window width (one f32 bank)
LO = 32768           # int16 index limit for dma_gather


def _ceil_to(v, m):
    return -(-v // m) * m


def _pass_plan(streams, nch):
    """streams: per-core (j_local, i_local, alpha) for one pass (i-sorted).
    Pads every core to nch chunks; returns J, IL, AL [NCORES, nch*128] and
    anchors (len nch), spread."""
    npos = nch * CHUNK
    J = np.zeros((NCORES, npos), np.int32)
    IL = np.zeros((NCORES, npos), np.int32)
    AL = np.zeros((NCORES, npos), np.float32)
    for m, (j, il, al) in enumerate(streams):
        n = len(j)
        J[m, :n] = j
        IL[m, :n] = il
        AL[m, :n] = al
        IL[m, n:] = il[-1] if n > 0 else 0
    ILc = IL.reshape(NCORES, nch, CHUNK)
    anchors = ILc.min(axis=2).min(axis=0).astype(np.int64)
    anchors = np.maximum.accumulate(anchors)
    spread = int((ILc.max(axis=2).max(axis=0) - anchors).max()) + 1
    return J, IL, AL, anchors, spread


def _regions_for(anchors, A, base_c):
    regions = []
    if len(anchors) == 0:
        return regions
    c0, r0 = 0, int(anchors[0])
    for c in range(len(anchors)):
        if int(anchors[c]) + A - r0 > PSUM_W:
            regions.append((base_c + c0, base_c + c, r0))
            c0, r0 = c, int(anchors[c])
    regions.append((base_c + c0, base_c + len(anchors), r0))
    return regions


def _make_plan(x, alpha_ij, idx_i, idx_j, W):
    n_atoms, F = x.shape
    assert F == 128 and n_atoms % NCORES == 0
    apc = n_atoms // NCORES  # atoms per core

    idx_i = np.asarray(idx_i)
    idx_j = np.asarray(idx_j)
    alpha_ij = np.asarray(alpha_ij, dtype=np.float32)

    bounds = np.searchsorted(idx_i, np.arange(NCORES + 1) * apc)
    lo_streams, hi_streams = [], []
    nch_lo = nch_hi = 0
    for m in range(NCORES):
        e0, e1 = int(bounds[m]), int(bounds[m + 1])
        j = idx_j[e0:e1].astype(np.int32)
        il = (idx_i[e0:e1] - m * apc).astype(np.int32)
        al = alpha_ij[e0:e1]
        sel = j < LO
        lo_streams.append((j[sel], il[sel], al[sel]))
        hi_streams.append((j[~sel] - LO, il[~sel], al[~sel]))
        nch_lo = max(nch_lo, len(lo_streams[-1][0]))
        nch_hi = max(nch_hi, len(hi_streams[-1][0]))

    nch_lo = _ceil_to(_ceil_to(nch_lo, CHUNK) // CHUNK, GB) if nch_lo else 0
    nch_hi = _ceil_to(_ceil_to(nch_hi, CHUNK) // CHUNK, GB) if nch_hi else 0
    nch = nch_lo + nch_hi

    Jl, ILl, ALl, anch_lo, spread_lo = _pass_plan(lo_streams, nch_lo)
    if nch_hi:
        Jh, ILh, ALh, anch_hi, spread_hi = _pass_plan(hi_streams, nch_hi)
    else:
        Jh = np.zeros((NCORES, 0), np.int32)
        ILh = np.zeros((NCORES, 0), np.int32)
        ALh = np.zeros((NCORES, 0), np.float32)
        anch_hi, spread_hi = np.zeros(0, np.int64), 1

    A_lo = _ceil_to(spread_lo, 8)
    A_hi = _ceil_to(spread_hi, 8) if nch_hi else 8
    A = max(A_lo, A_hi)
    assert A <= PSUM_W, f"chunk atom spread {A} exceeds psum window {PSUM_W}"

    regions = (_regions_for(anch_lo, A_lo, 0) +
               _regions_for(anch_hi, A_hi, nch_lo))
    anchors = np.concatenate([anch_lo, anch_hi])

    out_w = _ceil_to(apc, PSUM_W)
    st_w = max(max(r0 for _, _, r0 in regions) + PSUM_W, out_w)

    J = np.concatenate([Jl, Jh], axis=1)
    IL = np.concatenate([ILl, ILh], axis=1)
    AL = np.concatenate([ALl, ALh], axis=1)
    idxl = IL - np.repeat(anchors[None, :], CHUNK, axis=0).T.reshape(1, nch * CHUNK)

    def to2d(arr, dt):
        # position (c*128 + p) -> [p, c]
        return np.ascontiguousarray(arr.reshape(-1, CHUNK).T).astype(dt)

    def wrap16(arr):
        # position k -> partition k%16 (replicated x8), word k//16
        w16 = np.ascontiguousarray(arr.reshape(-1, 16).T)  # [16, npos/16]
        return np.tile(w16, (8, 1)).astype(np.int16)

    in_maps = []
    xb = np.asarray(x, dtype=np.float32).astype(BF16)
    wt = np.ascontiguousarray(np.asarray(W, dtype=np.float32).T)
    iota = np.tile(np.arange(A, dtype=np.float32).astype(BF16), (128, 1))
    for m in range(NCORES):
        in_maps.append({
            "xb": xb,
            "gidx": wrap16(J[m]),
            "idxl": to2d(idxl[m].astype(np.float32), BF16),
            "alp": to2d(AL[m], BF16),
            "iota": iota,
            "wt": wt,
        })

    # per-batch selector width: max chunk spread within each GB-batch
    ILc_all = np.concatenate(
        [ILl.reshape(NCORES, nch_lo, CHUNK),
         ILh.reshape(NCORES, nch_hi, CHUNK)], axis=1)
    chunk_max = ILc_all.max(axis=2).max(axis=0)  # [nch]
    A_b = []
    for b in range(nch // GB):
        sl = slice(b * GB, (b + 1) * GB)
        sp = int((chunk_max[sl] - anchors[sl]).max()) + 1
        A_b.append(min(_ceil_to(sp, 8), A))

    return dict(
        n_atoms=n_atoms, F=F, apc=apc, nch=nch, nch_lo=nch_lo, A=A,
        A_lo=A_lo, A_hi=A_hi, A_b=A_b,
        anchors=[int(a) for a in anchors], regions=regions,
        out_w=out_w, st_w=st_w, in_maps=in_maps,
    )


def _build_nc(plan, repeat=1, parts=("gather", "m", "pe"), hw_loop=0,
              gbufs=6, mbufs=4, pbufs=2):
    do_gather = "gather" in parts
    do_m = "m" in parts
    do_pe = "pe" in parts
    F = plan["F"]
    nch, nch_lo, A = plan["nch"], plan["nch_lo"], plan["A"]
    regions, anchors = plan["regions"], plan["anchors"]
    out_w, st_w = plan["out_w"], plan["st_w"]
    n_atoms = plan["n_atoms"]
    f32 = mybir.dt.float32
    bf16 = mybir.dt.bfloat16
    WPC = CHUNK // 16  # gidx words per chunk

    nc = bacc.Bacc("TRN2", target_bir_lowering=False, debug=False,
                   num_devices=NCORES, num_swdge_queues=4)

    xb_d = nc.dram_tensor("xb", [n_atoms, F], bf16, kind="ExternalInput")
    gidx_d = nc.dram_tensor("gidx", [128, nch * WPC], mybir.dt.int16,
                            kind="ExternalInput")
    idxl_d = nc.dram_tensor("idxl", [128, nch], bf16, kind="ExternalInput")
    alp_d = nc.dram_tensor("alp", [128, nch], bf16, kind="ExternalInput")
    iota_d = nc.dram_tensor("iota", [128, A], bf16, kind="ExternalInput")
    wt_d = nc.dram_tensor("wt", [F, F], f32, kind="ExternalInput")
    yt_d = nc.dram_tensor("yT", [128, out_w], f32, kind="ExternalOutput")

    region_start = {c0: (c0, c1, r0) for (c0, c1, r0) in regions}
    region_end = {c1 - 1: (c0, c1, r0) for (c0, c1, r0) in regions}

    with TileContext(nc) as tc:
        with (
            tc.tile_pool(name="const", bufs=1) as cpool,
            tc.tile_pool(name="gp", bufs=gbufs) as gpool,
            tc.tile_pool(name="mp", bufs=mbufs) as mpool,
            tc.tile_pool(name="pp", bufs=pbufs, space="PSUM") as ppool,
            tc.tile_pool(name="yp", bufs=2, space="PSUM") as ypool,
            tc.tile_pool(name="yo", bufs=2) as yopool,
        ):
            gidx_s = cpool.tile([128, nch * WPC], mybir.dt.int16)
            idxl_s = cpool.tile([128, nch], bf16)
            alp_s = cpool.tile([128, nch], bf16)
            iota_s = cpool.tile([128, A], bf16)
            wt_s = cpool.tile([F, F], f32)
            st_s = cpool.tile([128, st_w], f32)

            nc.sync.dma_start(gidx_s[:, :], gidx_d[:, :])
            nc.sync.dma_start(idxl_s[:, :], idxl_d[:, :])
            nc.sync.dma_start(alp_s[:, :], alp_d[:, :])
            nc.sync.dma_start(iota_s[:, :], iota_d[:, :])
            nc.sync.dma_start(wt_s[:, :], wt_d[:, :])
            nc.vector.memset(st_s[:, :], 0.0)

            if not do_gather:
                g_const = gpool.tile([128, GB, F], bf16, tag="g_t")
                nc.vector.memset(g_const[:, :, :], 0.0)
            if not do_m:
                m_const = mpool.tile([128, GB, plan["A"]], bf16, tag="m_t")
                nc.vector.memset(m_const[:, :, :], 0.0)

            import contextlib
            loop_cm = (tc.For_i(0, hw_loop, 1) if hw_loop
                       else contextlib.nullcontext())
            with loop_cm:
                psum_t = None
                cur_r0 = 0
                call_no = 0
                for b in list(range(nch // GB)) * repeat:
                    if b * GB < nch_lo:
                        src = xb_d[0:min(n_atoms, LO), :]
                    else:
                        src = xb_d[LO:n_atoms, :]
                    Ab = plan["A_b"][b]
                    if do_gather:
                        g_t = gpool.tile([128, GB, F], bf16, tag="g_t")
                        for gc in range(GB // GCALL):
                            c0w = (b * GB + gc * GCALL) * WPC
                            nc.gpsimd.dma_gather(
                                g_t[:, gc * GCALL:(gc + 1) * GCALL, :], src,
                                gidx_s[:, c0w:c0w + GCALL * WPC],
                                GCALL * CHUNK, GCALL * CHUNK, F,
                                queue_num=call_no % 4,
                            )
                            call_no += 1
                    else:
                        g_t = g_const
                    if do_m:
                        m_t = mpool.tile([128, GB, Ab], bf16, tag="m_t")
                        idxl_b = (idxl_s[:, b * GB:(b + 1) * GB]
                                  .unsqueeze(2).broadcast_to((128, GB, Ab)))
                        alp_b = (alp_s[:, b * GB:(b + 1) * GB]
                                 .unsqueeze(2).broadcast_to((128, GB, Ab)))
                        iota_bb = (iota_s[:, :Ab]
                                   .unsqueeze(1).broadcast_to((128, GB, Ab)))
                        nc.any.tensor_tensor(m_t[:, :, :], iota_bb, idxl_b,
                                             op=mybir.AluOpType.is_equal)
                        nc.any.tensor_tensor(m_t[:, :, :], m_t[:, :, :],
                                             alp_b, op=mybir.AluOpType.mult)
                    else:
                        m_t = m_const

                    if do_pe:
                        for cl in range(GB):
                            c = b * GB + cl
                            if c in region_start:
                                _, _, cur_r0 = region_start[c]
                                psum_t = ppool.tile([128, PSUM_W], f32)
                                nc.vector.memset(psum_t[:, :], 0.0)
                            off = anchors[c] - cur_r0
                            nc.tensor.matmul(
                                psum_t[:, off:off + Ab],
                                g_t[:, cl, :],
                                m_t[:, cl, :Ab],
                                start=False,
                                stop=(c in region_end),
                                skip_group_check=True,
                            )
                            if c in region_end:
                                _, _, r0 = region_end[c]
                                nc.any.tensor_add(st_s[:, r0:r0 + PSUM_W],
                                                  psum_t[:, :],
                                                  st_s[:, r0:r0 + PSUM_W])

            for ob in range(out_w // PSUM_W):
                yp_t = ypool.tile([128, PSUM_W], f32)
                nc.tensor.matmul(
                    yp_t[:, :], wt_s[:, :],
                    st_s[:, ob * PSUM_W:(ob + 1) * PSUM_W],
                    start=True, stop=True,
                )
                yo_t = yopool.tile([128, PSUM_W], f32)
                nc.any.tensor_copy(yo_t[:, :], yp_t[:, :])
                nc.sync.dma_start(yt_d[:, ob * PSUM_W:(ob + 1) * PSUM_W],
                                  yo_t[:, :])

    nc.compile()
    return nc


def kernel(x, alpha_ij, idx_i, idx_j, W):
    plan = _make_plan(x, alpha_ij, idx_i, idx_j, W)
    nc = _build_nc(plan)
    res = run_bass_kernel_spmd(nc, plan["in_maps"], list(range(NCORES)))
    apc = plan["apc"]
    y = np.empty((plan["n_atoms"], plan["F"]), np.float32)
    for m in range(NCORES):
        y[m * apc:(m + 1) * apc] = res.results[m]["yT"][:, :apc].T
    return y
